# revision 20
# baseline (speedup 1.0000x reference)
"""Trainium2 Bass kernel for nn_DNN_sym_10101763080772 (moe_routing).

Network (all-linear, batch-1):
    g1  = x @ W1.T + b1          [128, 3]
    g12 = x @ W12.T + b12        [128, 3]
    g   = where(atom_list == 1, g1, g12)
    d   = (g.T @ x).reshape(9)
    h0  = d  @ Wl0.T + bl0       [8192]
    h1  = h0 @ Wl1.T + bl1       [8192]
    h2  = h1 @ Wl2.T + bl2       [8192]
    out = h2 @ Wo.T  + bo        [3]

Sharding over 8 cores (tensor parallel, no collectives):
  - embed/routing stage + h0 replicated on every core (tiny).
  - Wl1 row-sharded: core i computes h1[1024*i : 1024*(i+1)] exactly.
  - Wl2 column-sharded with the same slice: core i computes a partial h2.
  - Because the network is linear past that point, each core applies Wo to
    its partial h2 and returns a partial [3]; the host sums the 8 partials.
  - bl2 / bo are folded in on core 0 only (other cores get zero tensors).

All big matmuls use the "weights stationary, vector moving (N=1)"
orientation so every activation stays partition-major [128, C]; no
transposes are needed. Weights are pre-tiled on the host into
[128, 65536] slabs whose free dim is (mtile, ktile, m)-major, so the
kernel streams them with large contiguous DMAs straight into SBUF lhsT
tiles (sync-engine HWDGE ring).

v2 critical-path fix (measured on the v1 trace): the 16 DMA engines
round-robin packets across queues, so the v1 constants blob + int32
atom_list on the scalar ring trickled in behind the weight firehose and
h0 was not ready until ~40us; from there the whole stream ran
consumer-paced instead of bandwidth-paced (~420 GB/s for the first
75us, collapsing afterwards). v2 therefore:
  - precomputes the routing mask on the host (f32), killing the int32
    DMA and the is_equal step entirely;
  - sends blob4 + the early blob (x, mask, bl0, Wl0) FIRST on the sync
    ring, ahead of the weight chunks, so they land by ~10us;
  - keeps only the late constants (bl1, bl2, Wo, bo) on the scalar ring
    where trickling is harmless (first use ~50us);
  - reduces+broadcasts d in a single matmul against a memset all-ones
    [128,128] tile (no DMA, no ones row/col in the blob).
h0 is computed on the Vector engine (exact f32) to keep the Tensor
engine free for the streamed layers.
"""

import os
import sys

import numpy as np

if "/opt/trn_rl_repo" not in sys.path:
    sys.path.insert(0, "/opt/trn_rl_repo")

N_CORES = 8
NA = 128           # atoms
D = 8192           # hidden width
SH = D // N_CORES  # 1024 rows/cols per core

# "f32" (exact), "bf16" (half the HBM traffic), "f32r" (full-rate fp32 matmul)
BIG_DT = os.environ.get("KERNEL_DTYPE", "bf16")

# early blob (sync ring, IN FRONT of the weight chunks: the in-order DGE
# queue guarantees it lands before chunk 0 with zero contention) — f32.
# Rows 0-3 of the trailing columns carry the old blob4 (xT/ones row, W1/W12
# aug) so everything early is ONE dma.
_E_X = 0          # [*, 0:3]    x
_E_MASK = 3       # [*, 3:4]    routing mask (atom_list == 1) as f32
_E_ONES = 4       # [*, 4:5]    ones column (rhs of the bo-fold matmul)
_E_BL0 = 5        # [*, 5:69]   bl0 partition-major
_E_WL0 = 69       # [*, 69:645] Wl0 k-major [p, k*64+c]
_E_B4 = 645       # [0:4, 645:779] xT(128) | W1aug(3) | W12aug(3)
_E_W = 779

# late blob (scalar ring, trickles behind the weight stream) offsets, f32
_L_BL1 = 0        # [*, 0:8]    bl1 shard partition-major
_L_BL2 = 8        # [*, 8:72]   bl2 (core0) partition-major
_L_WOT = 72       # [*, 72:264] Wo tiled [p, c*3+m]
_L_WOBO = 264     # [*, 264:267] outer(ones,bo)/128: folds bo into the qp chain
_L_W = 267

_session = {}


def _build(big_dt_name):
    import concourse.bass as bass
    import concourse.mybir as mybir
    import concourse.tile as tile
    from concourse import bacc

    f32 = mybir.dt.float32
    big_dt = {
        "f32": mybir.dt.float32,
        "f32r": mybir.dt.float32r,
        "bf16": mybir.dt.bfloat16,
    }[big_dt_name]
    # ~4 MB streamed chunks. The full chunks rotate through 4 buffers; the
    # taper chunks get dedicated one-shot tiles so their DMA triggers fire as
    # soon as the sequencer reaches them (a rotating buffer would gate each
    # trigger on a full-chunk consumption period ~11us and let the DGE run
    # dry before the last taper bytes — measured on the v3 trace).
    chunk_f = 16384 if big_dt_name == "bf16" else 8192
    n_bufs = 4
    n_chunks = 65536 // chunk_f
    tiles_per_chunk = chunk_f // 128

    nc = bacc.Bacc("TRN2", target_bir_lowering=False, debug=False)

    eblob_d = nc.dram_tensor("eblob", [128, _E_W], f32, kind="ExternalInput")
    lblob_d = nc.dram_tensor("lblob", [128, _L_W], f32, kind="ExternalInput")
    l1w_d = nc.dram_tensor("l1w", [128, 65536], big_dt, kind="ExternalInput")
    l2w_d = nc.dram_tensor("l2w", [128, 65536], big_dt, kind="ExternalInput")
    q_d = nc.dram_tensor("q", [3, 1], f32, kind="ExternalOutput")

    add = mybir.AluOpType.add
    sub = mybir.AluOpType.subtract
    mult = mybir.AluOpType.mult

    with tile.TileContext(nc) as tc:
        with (
            tc.tile_pool(name="const", bufs=1) as cp,
            tc.tile_pool(name="work", bufs=1) as wk,
            tc.tile_pool(name="wstream", bufs=n_bufs) as ws,
            tc.tile_pool(name="ps", bufs=1, space=bass.MemorySpace.PSUM) as pp,
        ):
            # ---- constants. The early blob rides the sync ring IN FRONT of
            # the weight chunks (in-order queue -> lands first, uncontended);
            # the late blob trickles on the scalar ring (first use ~50us).
            eb = cp.tile([128, _E_W], f32)
            lb = cp.tile([128, _L_W], f32)
            nc.sync.dma_start(out=eb[:], in_=eblob_d[:])
            nc.scalar.dma_start(out=lb[:], in_=lblob_d[:])

            x_sb = eb[:, _E_X : _E_X + 3]
            mask = eb[:, _E_MASK : _E_MASK + 1]
            ones_col = eb[:, _E_ONES : _E_ONES + 1]
            bl0p = eb[:, _E_BL0 : _E_BL0 + 64]
            bl1p = lb[:, _L_BL1 : _L_BL1 + 8]
            bl2p = lb[:, _L_BL2 : _L_BL2 + 64]
            wot = lb[:, _L_WOT : _L_WOT + 192]
            wobo = lb[:, _L_WOBO : _L_WOBO + 3]
            xTa = eb[0:4, _E_B4 : _E_B4 + 128]
            w1aug = eb[0:4, _E_B4 + 128 : _E_B4 + 131]
            w12aug = eb[0:4, _E_B4 + 131 : _E_B4 + 134]

            # all-ones lhsT for the fused colsum+broadcast matmul (no DMA)
            ones128 = wk.tile([128, 128], f32)
            nc.gpsimd.memset(ones128[:], 1.0)

            # ---- routed embedding: g = g12 + mask*(g1 - g12) ----
            g1p = pp.tile([NA, 3], f32)
            g12p = pp.tile([NA, 3], f32)
            nc.tensor.matmul(g1p[:], xTa, w1aug, start=True, stop=True)
            nc.tensor.matmul(g12p[:], xTa, w12aug, start=True, stop=True)

            g12_sb = wk.tile([NA, 3], f32)
            nc.vector.tensor_copy(g12_sb[:], g12p[:])
            diff = wk.tile([NA, 3], f32)
            nc.vector.tensor_tensor(diff[:], g1p[:], g12_sb[:], sub)
            g_sb = wk.tile([NA, 3], f32)
            nc.vector.scalar_tensor_tensor(g_sb[:], diff[:], mask, g12_sb[:], mult, add)

            # ---- d = vec(g.T @ x), reduced over atoms AND broadcast to all
            # 128 partitions in one matmul against the all-ones lhsT
            gx = wk.tile([NA, 9], f32)
            for a in range(3):
                nc.vector.tensor_scalar_mul(
                    gx[:, 3 * a : 3 * a + 3], x_sb, g_sb[:, a : a + 1]
                )
            dbp = pp.tile([128, 9], f32)
            nc.tensor.matmul(dbp[:], ones128[:], gx[:], start=True, stop=True)
            dbc = wk.tile([128, 9], f32)
            nc.vector.tensor_copy(dbc[:], dbp[:])

            # ---- h0 = Wl0 @ d + bl0 on the Vector engine, [128, 64] ----
            acc_a = wk.tile([128, 64], f32)
            acc_b = wk.tile([128, 64], f32)
            h0 = wk.tile([128, 64], big_dt)
            cur, nxt = acc_a, acc_b
            nc.vector.scalar_tensor_tensor(
                cur[:], eb[:, _E_WL0 : _E_WL0 + 64], dbc[:, 0:1], bl0p, mult, add
            )
            for k in range(1, 9):
                dst = h0 if k == 8 else nxt
                nc.vector.scalar_tensor_tensor(
                    dst[:],
                    eb[:, _E_WL0 + 64 * k : _E_WL0 + 64 * (k + 1)],
                    dbc[:, k : k + 1],
                    cur[:],
                    mult,
                    add,
                )
                cur, nxt = nxt, cur

            # ---- layer 1 (row shard): h1_i = Wl1[rows] @ h0 + bl1[rows] ----
            # slab free index = mtile*8192 + ktile*128 + m ; tile t = mtile*64+ktile
            h1p = pp.tile([128, 8], f32)
            for c in range(n_chunks):
                wt = ws.tile([128, chunk_f], big_dt, tag="wchunk")
                nc.sync.dma_start(out=wt[:], in_=l1w_d[:, c * chunk_f : (c + 1) * chunk_f])
                for j in range(tiles_per_chunk):
                    t = c * tiles_per_chunk + j
                    mt, kt = divmod(t, 64)
                    nc.tensor.matmul(
                        h1p[:, mt : mt + 1],
                        wt[:, j * 128 : (j + 1) * 128],
                        h0[:, kt : kt + 1],
                        start=(kt == 0),
                        stop=(kt == 63),
                    )
            h1 = wk.tile([128, 8], big_dt)
            nc.vector.tensor_tensor(h1[:], h1p[:], bl1p, add)

            # ---- layer 2 (col shard): p2 = Wl2[:, cols] @ h1_i (+ bl2 core0)
            # slab free index = mtile2*1024 + kchunk*128 + m ; tile t = mtile2*8+kchunk
            # The final q = Wo @ p2 contraction is interleaved per chunk so no
            # work is left after the last weight byte lands; p2 PSUM ping-pongs
            # between two banks so the evacuating vector reads never collide
            # with the next chunk's matmul writes. The last chunks taper off in
            # size for the same reason.
            # taper stops at half-chunks: rows below ~16KB/partition fall off
            # a DGE descriptor-efficiency cliff (last 0.5MB chunks measured at
            # 17-40 GB/s vs ~427 GB/s for 32KB rows), costing far more than
            # the shorter compute tail saves
            full = tiles_per_chunk
            taper = [full // 2, full // 2]
            l2_chunks = [full] * (n_chunks - 1) + taper
            p2pa = pp.tile([128, full // 8], f32)
            p2pb = pp.tile([128, full // 8], f32)
            p2sb = wk.tile([128, 64], f32)
            qp = pp.tile([3, 1], f32)
            # bo lands in the qp accumulator up front (wobo.T @ ones = bo on
            # core 0, zero elsewhere), so nothing but a PSUM read remains
            # after the last Wo matmul
            nc.tensor.matmul(qp[:], wobo, ones_col, start=True, stop=False)
            t0 = 0
            for ci, ntiles in enumerate(l2_chunks):
                if ntiles == full:
                    wt = ws.tile([128, ntiles * 128], big_dt, tag="wchunk")
                else:
                    # dedicated one-shot buffer (unique tag per chunk!): the
                    # trigger fires in program order, no watermark wait
                    wt = wk.tile([128, ntiles * 128], big_dt, tag=f"tap{ci}")
                nc.sync.dma_start(
                    out=wt[:], in_=l2w_d[:, t0 * 128 : (t0 + ntiles) * 128]
                )
                p2p = p2pa if ci % 2 == 0 else p2pb
                mt0 = t0 // 8
                nmt = ntiles // 8
                for j in range(ntiles):
                    t = t0 + j
                    mt, kc = divmod(t, 8)
                    nc.tensor.matmul(
                        p2p[:, mt - mt0 : mt - mt0 + 1],
                        wt[:, j * 128 : (j + 1) * 128],
                        h1[:, kc : kc + 1],
                        start=(kc == 0),
                        stop=(kc == 7),
                    )
                nc.vector.tensor_tensor(
                    p2sb[:, mt0 : mt0 + nmt],
                    p2p[:, 0:nmt],
                    bl2p[:, mt0 : mt0 + nmt],
                    add,
                )
                for ch in range(mt0, mt0 + nmt):
                    nc.tensor.matmul(
                        qp[:],
                        wot[:, ch * 3 : (ch + 1) * 3],
                        p2sb[:, ch : ch + 1],
                        start=False,
                        stop=(ch == 63),
                    )
                t0 += ntiles

            q_sb = wk.tile([3, 1], f32)
            nc.vector.tensor_copy(q_sb[:], qp[:])
            nc.sync.dma_start(out=q_d[:], in_=q_sb[:])

    nc.compile()
    return nc


def _prep_in_maps(inputs, big_dt_name):
    import ml_dtypes

    big_np = np.dtype(ml_dtypes.bfloat16) if big_dt_name == "bf16" else np.float32

    f = lambda k: np.asarray(inputs[k], np.float32)
    x = f("x")
    W1, b1, W12, b12 = f("W1"), f("b1"), f("W12"), f("b12")
    Wl0, bl0 = f("Wl0"), f("bl0")
    Wl1, bl1 = f("Wl1"), f("bl1")
    Wl2, bl2 = f("Wl2"), f("bl2")
    Wo, bo = f("Wo"), f("bo")
    atom = np.asarray(inputs["atom_list"], np.int32)

    eblob = np.zeros((128, _E_W), np.float32)
    eblob[:, _E_X : _E_X + 3] = x
    eblob[:, _E_MASK] = (atom == 1).astype(np.float32)
    eblob[:, _E_ONES] = 1.0
    eblob[:, _E_BL0 : _E_BL0 + 64] = bl0.reshape(64, 128).T
    # Wl0 k-major: [p, k*64 + c] = Wl0[c*128+p, k]
    eblob[:, _E_WL0 : _E_WL0 + 576] = (
        Wl0.reshape(64, 128, 9).transpose(1, 2, 0).reshape(128, 576)
    )
    eblob[0:3, _E_B4 : _E_B4 + 128] = x.T
    eblob[3, _E_B4 : _E_B4 + 128] = 1.0
    eblob[0:3, _E_B4 + 128 : _E_B4 + 131] = W1.T
    eblob[3, _E_B4 + 128 : _E_B4 + 131] = b1
    eblob[0:3, _E_B4 + 131 : _E_B4 + 134] = W12.T
    eblob[3, _E_B4 + 131 : _E_B4 + 134] = b12

    lblob = np.zeros((128, _L_W), np.float32)
    lblob[:, _L_BL2 : _L_BL2 + 64] = bl2.reshape(64, 128).T  # zeroed for cores 1-7
    lblob[:, _L_WOT : _L_WOT + 192] = (
        Wo.reshape(3, 64, 128).transpose(2, 1, 0).reshape(128, 192)
    )
    lblob[:, _L_WOBO : _L_WOBO + 3] = bo[None, :] / 128.0

    Wl1b = Wl1.astype(big_np)  # cast before relayout: halves the shuffle bytes
    Wl2b = Wl2.astype(big_np)
    in_maps = []
    for i in range(N_CORES):
        rows = slice(SH * i, SH * (i + 1))
        l1w = np.ascontiguousarray(
            Wl1b[rows].reshape(8, 128, 64, 128).transpose(3, 0, 2, 1).reshape(128, 65536)
        )
        l2w = np.ascontiguousarray(
            Wl2b[:, rows].reshape(64, 128, 8, 128).transpose(3, 0, 2, 1).reshape(128, 65536)
        )
        lbi = lblob.copy()
        lbi[:, _L_BL1 : _L_BL1 + 8] = bl1[rows].reshape(8, 128).T
        if i != 0:
            lbi[:, _L_BL2 : _L_BL2 + 64] = 0.0
            lbi[:, _L_WOBO : _L_WOBO + 3] = 0.0
        in_maps.append({"eblob": eblob, "lblob": lbi, "l1w": l1w, "l2w": l2w})
    return in_maps


def _install_profile_shim():
    """Make trace=True work under axon: provide the antenv.axon_hooks
    registry this container's antenv stub lacks, wired to the ctypes NTFF
    profiler from trn_agent_boot."""
    import types

    try:
        from antenv.axon_hooks import get_axon_ntff_profile_hook  # noqa: F401
        return
    except ImportError:
        pass
    try:
        import antenv
        from trn_agent_boot.trn_boot import _ntff_profile_via_ctypes

        mod = types.ModuleType("antenv.axon_hooks")
        holder = {"h": None}
        mod.set_axon_ntff_profile_hook = lambda h: holder.__setitem__("h", h)
        mod.get_axon_ntff_profile_hook = lambda: holder["h"]
        sys.modules["antenv.axon_hooks"] = mod
        antenv.axon_hooks = mod
        mod.set_axon_ntff_profile_hook(
            _ntff_profile_via_ctypes("/opt/axon/libaxon_pjrt.so")
        )
    except Exception as e:  # profiling is best-effort only
        print(f"profile shim unavailable: {e}")


def kernel(**inputs) -> np.ndarray:
    from concourse import bass_utils

    big = BIG_DT
    if big not in _session:
        _session[big] = _build(big)
    nc = _session[big]

    in_maps = _prep_in_maps(inputs, big)
    trace = os.environ.get("KERNEL_TRACE", "0") == "1"
    if trace:
        _install_profile_shim()
    res = bass_utils.run_bass_kernel_spmd(
        nc, in_maps, core_ids=list(range(N_CORES)), trace=trace
    )
    if trace and res.exec_time_ns is not None:
        print(f"HW exec time: {res.exec_time_ns} ns")
        kernel.last_exec_time_ns = res.exec_time_ns
    kernel.last_results = res
    out = np.zeros(3, np.float64)
    for r in res.results:
        out += r["q"][:, 0].astype(np.float64)
    return out.astype(np.float32)
